# revision 2
# baseline (speedup 1.0000x reference)
"""Trainium2 Bass kernel for a 2-layer LSTM decoder (nn_Decoder).

Reference computation (per step, 30 steps):
    L0: gates = W_ih0 @ x + W_hh0 @ h0 + b;  LSTM cell -> h0', c0'
    L1: gates = W_ih1 @ h0' + W_hh1 @ h1 + b; LSTM cell -> h1', c1'
    out = W_hp @ h1' + b_hp ;  x_next = out

Structure:
 1. Step 0 (the only step that consumes the observed x) runs on the HOST in
    numpy; the device receives the post-step-0 state and runs 29 UNIFORM
    steps.  For t >= 1, x_t = W_hp @ h1_{t-1} + b_hp folds into an effective
    weight on h1_{t-1}: W_eff = W_ih0 @ W_hp, b0_eff = b_ih0 + b_hh0 +
    W_ih0 @ b_hp.  This removes the K=3 matmul entirely (lhsT/rhs with
    partition-dim 3 wedges the exec unit: NRT_EXEC_UNIT_UNRECOVERABLE).
 2. Gate biases are injected into PSUM by packed K=1 rank-1 matmuls
    (bias-row x ones) at tile_position=(32j,0) - the 4 row-group matmuls run
    concurrently in the PE array, so the whole bias injection costs ~one
    matmul slot.  Sigmoid over the [i|f|o] chunks then runs as ONE
    activation op spanning 3 PSUM banks.
 3. Gate chunk order is host-permuted from PyTorch's (i,f,g,o) to (i,f,o,g)
    so i,f,o land in adjacent PSUM banks.
 4. The projection W_hp @ h1 runs with W_hp^T as the STATIONARY operand
    (fixed weights, 3 columns -> negligible LDWEIGHTS), output [3, 512] per
    batch tile; b_hp is added on the host after gather.

Layout: state kept on-chip as [H=128 partitions, B_local free]; batch
data-parallel over 8 cores (1024 each), 2 tiles of 512 per core.
Matmuls run as float32r (fp32 data, full PE rate at N >= 256).
"""

import numpy as np

import concourse.bass as bass
import concourse.mybir as mybir
from concourse import bacc
from concourse.tile import TileContext

N_CORES = 8
B = 8192
BL = B // N_CORES  # 1024 per core
H = 128
PRED = 30
DSTEPS = PRED - 1   # 29 device steps; step 0 on host
NT = 2              # batch tiles per core
TN = BL // NT       # 512 = one PSUM bank of fp32

F32 = mybir.dt.float32
F32R = mybir.dt.float32r

_COMPILED = {}
_RUNNERS = {}


def build_bass(reps=1, loop=False):
    """29-step uniform LSTM decoder body.  loop=True wraps the body in a
    hardware For_i(0, reps) loop (same NEFF size for any reps) - used for
    timing; the graded path uses loop=False (straight-line, reps=1)."""
    nc = bacc.Bacc("TRN2", target_bir_lowering=False, debug=False)

    # Per-core state after host step 0
    d_h0 = nc.declare_dram_parameter("h0T", [H, BL], F32R, isOutput=False)
    d_h1 = nc.declare_dram_parameter("h1T", [H, BL], F32R, isOutput=False)
    d_c0 = nc.declare_dram_parameter("c0T", [H, BL], F32, isOutput=False)
    d_c1 = nc.declare_dram_parameter("c1T", [H, BL], F32, isOutput=False)
    # Replicated weights (gate chunk order i,f,o,g)
    d_weff = nc.declare_dram_parameter("weff", [H, 4 * H], F32R, isOutput=False)
    d_whh0 = nc.declare_dram_parameter("whh0", [H, 4 * H], F32R, isOutput=False)
    d_wih1 = nc.declare_dram_parameter("wih1", [H, 4 * H], F32R, isOutput=False)
    d_whh1 = nc.declare_dram_parameter("whh1", [H, 4 * H], F32R, isOutput=False)
    d_whp = nc.declare_dram_parameter("whpT", [H, 3], F32R, isOutput=False)
    # Bias rows: [128,128], row 32*j holds bias chunk j (order i,f,o,g)
    d_b0 = nc.declare_dram_parameter("b0", [H, H], F32R, isOutput=False)
    d_b1 = nc.declare_dram_parameter("b1", [H, H], F32R, isOutput=False)
    d_ones = nc.declare_dram_parameter("ones", [H, TN], F32R, isOutput=False)
    # Output: step t (device) = global step t+1; [3, BL] = (W_hp @ h1)^  (no bias)
    d_out = nc.declare_dram_parameter("preds", [DSTEPS, 3, BL], F32,
                                      isOutput=True)

    SIG = mybir.ActivationFunctionType.Sigmoid
    TANH = mybir.ActivationFunctionType.Tanh

    with TileContext(nc) as tc:
        with (
            tc.tile_pool(name="const", bufs=1) as cpool,
            tc.tile_pool(name="state", bufs=1) as spool,
            tc.tile_pool(name="work", bufs=3) as wpool,
            tc.tile_pool(name="outp", bufs=2) as opool_sb,
            tc.tile_pool(name="pq_pool", bufs=2, space="PSUM") as qpool,
            tc.tile_pool(name="pg_pool", bufs=2, space="PSUM") as gpool,
        ):
            # --- load constants / weights ---
            weff = cpool.tile([H, 4 * H], F32R)
            nc.sync.dma_start(out=weff[:], in_=d_weff[:])
            whh0 = cpool.tile([H, 4 * H], F32R)
            nc.sync.dma_start(out=whh0[:], in_=d_whh0[:])
            wih1 = cpool.tile([H, 4 * H], F32R)
            nc.sync.dma_start(out=wih1[:], in_=d_wih1[:])
            whh1 = cpool.tile([H, 4 * H], F32R)
            nc.sync.dma_start(out=whh1[:], in_=d_whh1[:])
            whpT = cpool.tile([H, 3], F32R)
            nc.sync.dma_start(out=whpT[:], in_=d_whp[:])
            b0 = cpool.tile([H, H], F32R)
            nc.sync.dma_start(out=b0[:], in_=d_b0[:])
            b1 = cpool.tile([H, H], F32R)
            nc.sync.dma_start(out=b1[:], in_=d_b1[:])
            ones = cpool.tile([H, TN], F32R)
            nc.sync.dma_start(out=ones[:], in_=d_ones[:])

            # --- state (persistent, updated in place) ---
            h_0 = spool.tile([H, BL], F32R)
            nc.sync.dma_start(out=h_0[:], in_=d_h0[:])
            h_1 = spool.tile([H, BL], F32R)
            nc.sync.dma_start(out=h_1[:], in_=d_h1[:])
            c_0 = spool.tile([H, BL], F32)
            nc.sync.dma_start(out=c_0[:], in_=d_c0[:])
            c_1 = spool.tile([H, BL], F32)
            nc.sync.dma_start(out=c_1[:], in_=d_c1[:])

            def emit_step(t):
                for li in (0, 1):
                    if li == 0:
                        bias, win, rhs_in = b0, weff, h_1
                        wrec, c_st, h_st = whh0, c_0, h_0
                    else:
                        bias, win, rhs_in = b1, wih1, h_0
                        wrec, c_st, h_st = whh1, c_1, h_1
                    for nt in range(NT):
                        sl = slice(nt * TN, (nt + 1) * TN)
                        # gate pre-activations: quad = [i|f|o] (3 banks), g
                        quad = qpool.tile([H, 3 * TN], F32, name="quad",
                                          tag="quad")
                        gq = gpool.tile([H, TN], F32, name="gq", tag="gp")

                        def bank(j):
                            return gq[:] if j == 3 else \
                                quad[:, j * TN:(j + 1) * TN]

                        for j in range(4):  # bias (rank-1, packed row-tiles)
                            nc.tensor.matmul(
                                bank(j),
                                bias[32 * j:32 * j + 1, :],
                                ones[32 * j:32 * j + 1, :],
                                start=True, stop=False,
                                tile_position=(32 * j, 0),
                            )
                        for j in range(4):  # input-side
                            js = slice(j * H, (j + 1) * H)
                            nc.tensor.matmul(bank(j), win[:, js],
                                             rhs_in[:, sl],
                                             start=False, stop=False)
                        for j in range(4):  # recurrent-side
                            js = slice(j * H, (j + 1) * H)
                            nc.tensor.matmul(bank(j), wrec[:, js],
                                             h_st[:, sl],
                                             start=False, stop=True)

                        # activations: one sigmoid over [i|f|o], tanh over g
                        sifo = wpool.tile([H, 3 * TN], F32, name="sifo",
                                          tag="sifo")
                        nc.scalar.activation(sifo[:], quad[:], SIG)
                        tg = wpool.tile([H, TN], F32, name="tg", tag="tg")
                        nc.scalar.activation(tg[:], gq[:], TANH)
                        u = wpool.tile([H, TN], F32, name="u", tag="u")
                        nc.vector.tensor_mul(u[:], sifo[:, 0:TN], tg[:])
                        v = wpool.tile([H, TN], F32, name="v", tag="v")
                        nc.vector.tensor_mul(v[:], sifo[:, TN:2 * TN],
                                             c_st[:, sl])
                        nc.vector.tensor_add(c_st[:, sl], u[:], v[:])
                        tch = wpool.tile([H, TN], F32, name="tch", tag="tch")
                        nc.scalar.activation(tch[:], c_st[:, sl], TANH)
                        nc.vector.tensor_mul(h_st[:, sl],
                                             sifo[:, 2 * TN:3 * TN], tch[:])

                # projection: po[3, TN] = W_hp @ h1 tile (fixed stationary whpT)
                out_sb = opool_sb.tile([3, BL], F32, name="out_sb",
                                       tag="out_sb")
                for nt in range(NT):
                    sl = slice(nt * TN, (nt + 1) * TN)
                    po = gpool.tile([3, TN], F32, name="po", tag="gp")
                    nc.tensor.matmul(po[:], whpT[:], h_1[:, sl],
                                     start=True, stop=True)
                    nc.vector.tensor_copy(out_sb[:, sl], po[:])
                nc.sync.dma_start(out=d_out[t], in_=out_sb[:])

            if loop:
                with tc.For_i(0, reps, 1,
                              hint_engines=(mybir.EngineType.PE,)):
                    for t in range(DSTEPS):
                        emit_step(t)
            else:
                for t in range(DSTEPS):
                    emit_step(t)

    nc.compile()
    return nc


def _get_compiled(reps=1, loop=False):
    key = (reps, loop)
    if key not in _COMPILED:
        _COMPILED[key] = build_bass(reps=reps, loop=loop)
    return _COMPILED[key]


# ---------------------------------------------------------------------------
# Cached-jit SPMD runner (no donation; inputs stay device-resident)
# ---------------------------------------------------------------------------

class _Runner:
    def __init__(self, nc, n_cores):
        import jax
        from jax.experimental.shard_map import shard_map
        from jax.sharding import Mesh, PartitionSpec
        from concourse import bass2jax

        bass2jax.install_neuronx_cc_hook()
        self.jax = jax
        self.n_cores = n_cores
        partition_name = (
            nc.partition_id_tensor.name if nc.partition_id_tensor else None
        )

        in_names, out_names, out_avals, zero_outs = [], [], [], []
        for alloc in nc.m.functions[0].allocations:
            if not isinstance(alloc, mybir.MemoryLocationSet):
                continue
            name = alloc.memorylocations[0].name
            if alloc.kind == "ExternalInput":
                if name != partition_name:
                    in_names.append(name)
            elif alloc.kind == "ExternalOutput":
                out_names.append(name)
                shape = tuple(alloc.tensor_shape)
                dtype = mybir.dt.np(alloc.dtype)
                out_avals.append(jax.core.ShapedArray(shape, dtype))
                zero_outs.append(np.zeros(shape, dtype))

        n_params = len(in_names)
        all_in_names = list(in_names) + list(out_names)
        if partition_name is not None:
            all_in_names.append(partition_name)

        self.in_names = in_names
        self.out_names = out_names
        self.out_avals = out_avals
        self.zero_outs = zero_outs

        def _body(*args):
            operands = list(args)
            if partition_name is not None:
                operands.append(bass2jax.partition_id_tensor())
            outs = bass2jax._bass_exec_p.bind(
                *operands,
                out_avals=tuple(out_avals),
                in_names=tuple(all_in_names),
                out_names=tuple(out_names),
                lowering_input_output_aliases=(),
                sim_require_finite=True,
                sim_require_nnan=True,
                nc=nc,
            )
            return tuple(outs)

        devices = jax.devices()[:n_cores]
        mesh = Mesh(np.asarray(devices), ("core",))
        n_outs = len(out_avals)
        in_specs = (PartitionSpec("core"),) * (n_params + n_outs)
        out_specs = (PartitionSpec("core"),) * n_outs
        self.fn = jax.jit(
            shard_map(_body, mesh=mesh, in_specs=in_specs,
                      out_specs=out_specs, check_rep=False),
            keep_unused=True,
        )
        self.sharding = jax.sharding.NamedSharding(mesh, PartitionSpec("core"))

    def put_inputs(self, in_maps):
        """Ship per-core inputs (concat on axis 0) + zero outputs to devices."""
        arrs = [
            np.concatenate([np.asarray(m[name]) for m in in_maps], axis=0)
            for name in self.in_names
        ]
        arrs += [
            np.zeros((self.n_cores * z.shape[0], *z.shape[1:]), z.dtype)
            for z in self.zero_outs
        ]
        return [self.jax.device_put(a, self.sharding) for a in arrs]

    def launch(self, dev_args):
        return self.fn(*dev_args)

    def fetch(self, out_arrs):
        res = []
        for c in range(self.n_cores):
            res.append({
                name: np.asarray(out_arrs[i]).reshape(
                    self.n_cores, *self.out_avals[i].shape)[c]
                for i, name in enumerate(self.out_names)
            })
        return res

    def run(self, in_maps, tries=5):
        import time as _time
        dev = self.put_inputs(in_maps)
        last = None
        for a in range(tries):
            try:
                outs = self.launch(dev)
                self.jax.block_until_ready(outs)
                return self.fetch(outs)
            except Exception as e:  # transient tunnel/terminal failures
                last = e
                _time.sleep(2.0)
        raise last


def get_runner(reps=1, loop=False):
    key = (reps, loop)
    if key not in _RUNNERS:
        _RUNNERS[key] = _Runner(_get_compiled(reps=reps, loop=loop), N_CORES)
    return _RUNNERS[key]


# ---------------------------------------------------------------------------
# Host-side prep: permutations, step 0, gather
# ---------------------------------------------------------------------------

PERM = (0, 1, 3, 2)  # PyTorch gate order (i,f,g,o) -> kernel order (i,f,o,g)


def _permg(w):
    """Permute gate-chunk rows of a [512, ...] weight (or [512] bias)."""
    return w.reshape(4, H, *w.shape[1:])[list(PERM)].reshape(w.shape)


def _bias_rows(b):
    """[512] bias -> [128,128] tile with chunk j (kernel order) at row 32j."""
    out = np.zeros((H, H), np.float32)
    bp = _permg(b)
    for j in range(4):
        out[32 * j] = bp[j * H:(j + 1) * H]
    return out


def _sigmoid(x):
    return 1.0 / (1.0 + np.exp(-x))


def _host_step0(obs_traj_rel, h0, c0, W_ih0, W_hh0, b_ih0, b_hh0,
                W_ih1, W_hh1, b_ih1, b_hh1, W_hp, b_hp):
    """Run reference step 0 in numpy fp32; return post-step state + out0."""
    f = np.float32
    x = obs_traj_rel[-1].astype(f)                      # [B, 3]
    hs = [h0[0].astype(f), h0[1].astype(f)]
    cs = [c0[0].astype(f), c0[1].astype(f)]
    params = ((W_ih0, W_hh0, b_ih0, b_hh0), (W_ih1, W_hh1, b_ih1, b_hh1))
    inp = x
    for l, (Wi, Wh, bi, bh) in enumerate(params):
        gates = inp @ Wi.T.astype(f) + hs[l] @ Wh.T.astype(f) \
            + bi.astype(f) + bh.astype(f)
        i, fg, g, o = np.split(gates, 4, axis=-1)
        c_new = _sigmoid(fg) * cs[l] + _sigmoid(i) * np.tanh(g)
        h_new = _sigmoid(o) * np.tanh(c_new)
        hs[l], cs[l] = h_new, c_new
        inp = h_new
    out0 = inp @ W_hp.T.astype(f) + b_hp.astype(f)      # [B, 3]
    return hs, cs, out0


def prep_inputs(obs_traj_rel, h0, c0, W_ih0, W_hh0, b_ih0, b_hh0,
                W_ih1, W_hh1, b_ih1, b_hh1, W_hp, b_hp):
    f = np.float32
    asc = np.ascontiguousarray

    hs, cs, out0 = _host_step0(
        obs_traj_rel, h0, c0, W_ih0, W_hh0, b_ih0, b_hh0,
        W_ih1, W_hh1, b_ih1, b_hh1, W_hp, b_hp)

    W_eff = (W_ih0.astype(f) @ W_hp.astype(f)).astype(f)       # [512, 128]
    b0_eff = (b_ih0 + b_hh0 + W_ih0 @ b_hp).astype(f)
    b1v = (b_ih1 + b_hh1).astype(f)

    shared = {
        "weff": asc(_permg(W_eff).T),             # [128, 512]
        "whh0": asc(_permg(W_hh0.astype(f)).T),
        "wih1": asc(_permg(W_ih1.astype(f)).T),
        "whh1": asc(_permg(W_hh1.astype(f)).T),
        "whpT": asc(W_hp.T.astype(f)),            # [128, 3]
        "b0": _bias_rows(b0_eff),
        "b1": _bias_rows(b1v),
        "ones": np.ones((H, TN), f),
    }

    h0T = hs[0].T       # [128, 8192]
    h1T = hs[1].T
    c0T = cs[0].T
    c1T = cs[1].T

    in_maps = []
    for c in range(N_CORES):
        bsl = slice(c * BL, (c + 1) * BL)
        m = dict(shared)
        m["h0T"] = asc(h0T[:, bsl].astype(f))
        m["h1T"] = asc(h1T[:, bsl].astype(f))
        m["c0T"] = asc(c0T[:, bsl].astype(f))
        m["c1T"] = asc(c1T[:, bsl].astype(f))
        in_maps.append(m)
    return in_maps, out0


def gather(res_results, out0, b_hp):
    # per-core preds [29, 3, 1024]; global step t = device step t-1
    dev = np.concatenate([res_results[c]["preds"] for c in range(N_CORES)],
                         axis=2)                       # [29, 3, 8192]
    preds = np.empty((PRED, B, 3), np.float32)
    preds[0] = out0
    preds[1:] = dev.transpose(0, 2, 1) + b_hp.astype(np.float32)
    return preds


def kernel(**inputs):
    inputs = {k: np.asarray(v) for k, v in inputs.items()}
    in_maps, out0 = prep_inputs(**inputs)
    r = get_runner(reps=1, loop=False)
    res = r.run(in_maps)
    return gather(res, out0, inputs["b_hp"])


# revision 6
# speedup vs baseline: 1.0784x; 1.0784x over previous
"""Trainium2 Bass kernel for a 2-layer LSTM decoder (nn_Decoder).

Reference computation (per step, 30 steps):
    L0: gates = W_ih0 @ x + W_hh0 @ h0 + b;  LSTM cell -> h0', c0'
    L1: gates = W_ih1 @ h0' + W_hh1 @ h1 + b; LSTM cell -> h1', c1'
    out = W_hp @ h1' + b_hp ;  x_next = out

Structure:
 1. Step 0 (the only step that consumes the observed x) runs on the HOST in
    numpy; the device receives the post-step-0 state and runs 29 UNIFORM
    steps.  For t >= 1, x_t = W_hp @ h1_{t-1} + b_hp folds into an effective
    weight on h1_{t-1}: W_eff = W_ih0 @ W_hp, b0_eff = b_ih0 + b_hh0 +
    W_ih0 @ b_hp.  This removes the K=3 matmul entirely (lhsT/rhs with
    partition-dim 3 wedges the exec unit: NRT_EXEC_UNIT_UNRECOVERABLE).
 2. Gate biases are injected into PSUM by packed K=1 rank-1 matmuls
    (bias-row x ones) at tile_position=(32j,0) - the 4 row-group matmuls run
    concurrently in the PE array, so the whole bias injection costs ~one
    matmul slot.
 3. Gate chunk order is host-permuted from PyTorch's (i,f,g,o) to (i,f,o,g);
    the g-chunk weights/bias are DOUBLED on the host, so tanh(g) =
    2*sigmoid(2g) - 1 and a single sigmoid over the whole [i|f|o|2g] 4-bank
    PSUM tile covers all four gates.  The cell math then runs as fused
    scalar_tensor_tensor ops:
        u2 = (sg - 0.5) * si            ( = si * tanh(g) / 2 )
        c' = (u2 * 2) + (sf * c)
    with the (sf * c) multiply offloaded to the otherwise-idle GPSIMD.
 4. The projection W_hp @ h1 runs with W_hp^T as the STATIONARY operand
    (fixed weights, 3 columns -> negligible LDWEIGHTS), output [3, 512] per
    batch tile; b_hp is added on the host after gather.

Layout: state kept on-chip as [H=128 partitions, B_local free]; batch
data-parallel over 8 cores (1024 each), 2 tiles of 512 per core.
Matmuls run as float32r (fp32 data, full PE rate at N >= 256).
"""

import numpy as np

import concourse.bass as bass
import concourse.mybir as mybir
from concourse import bacc
from concourse.tile import TileContext

N_CORES = 8
B = 8192
BL = B // N_CORES  # 1024 per core
H = 128
PRED = 30
DSTEPS = PRED - 1   # 29 device steps; step 0 on host
NT = 2              # batch tiles per core
TN = BL // NT       # 512 = one PSUM bank of fp32

F32 = mybir.dt.float32
F32R = mybir.dt.float32r

_COMPILED = {}
_RUNNERS = {}


def build_bass(reps=1, loop=False):
    """29-step uniform LSTM decoder body.  loop=True wraps the body in a
    hardware For_i(0, reps) loop (same NEFF size for any reps) - used for
    timing; the graded path uses loop=False (straight-line, reps=1)."""
    nc = bacc.Bacc("TRN2", target_bir_lowering=False, debug=False)

    # Per-core state after host step 0
    d_h0 = nc.declare_dram_parameter("h0T", [H, BL], F32R, isOutput=False)
    d_h1 = nc.declare_dram_parameter("h1T", [H, BL], F32R, isOutput=False)
    d_c0 = nc.declare_dram_parameter("c0T", [H, BL], F32, isOutput=False)
    d_c1 = nc.declare_dram_parameter("c1T", [H, BL], F32, isOutput=False)
    # Replicated weights (gate chunk order i,f,o,g)
    d_weff = nc.declare_dram_parameter("weff", [H, 4 * H], F32R, isOutput=False)
    d_whh0 = nc.declare_dram_parameter("whh0", [H, 4 * H], F32R, isOutput=False)
    d_wih1 = nc.declare_dram_parameter("wih1", [H, 4 * H], F32R, isOutput=False)
    d_whh1 = nc.declare_dram_parameter("whh1", [H, 4 * H], F32R, isOutput=False)
    d_whp = nc.declare_dram_parameter("whpT", [H, 3], F32R, isOutput=False)
    # Bias rows: [128,128], row 32*j holds bias chunk j (order i,f,o,g)
    d_b0 = nc.declare_dram_parameter("b0", [H, H], F32R, isOutput=False)
    d_b1 = nc.declare_dram_parameter("b1", [H, H], F32R, isOutput=False)
    d_ones = nc.declare_dram_parameter("ones", [H, TN], F32R, isOutput=False)
    # Output: step t (device) = global step t+1; [3, BL] = (W_hp @ h1)^  (no bias)
    d_out = nc.declare_dram_parameter("preds", [DSTEPS, 3, BL], F32,
                                      isOutput=True)

    SIG = mybir.ActivationFunctionType.Sigmoid
    TANH = mybir.ActivationFunctionType.Tanh

    with TileContext(nc) as tc:
        with (
            tc.tile_pool(name="const", bufs=1) as cpool,
            tc.tile_pool(name="state", bufs=1) as spool,
            tc.tile_pool(name="work", bufs=3) as wpool,
            tc.tile_pool(name="outp", bufs=2) as opool_sb,
            tc.tile_pool(name="pq_pool", bufs=2, space="PSUM") as qpool,
        ):
            # --- load constants / weights ---
            weff = cpool.tile([H, 4 * H], F32R)
            nc.sync.dma_start(out=weff[:], in_=d_weff[:])
            whh0 = cpool.tile([H, 4 * H], F32R)
            nc.sync.dma_start(out=whh0[:], in_=d_whh0[:])
            wih1 = cpool.tile([H, 4 * H], F32R)
            nc.sync.dma_start(out=wih1[:], in_=d_wih1[:])
            whh1 = cpool.tile([H, 4 * H], F32R)
            nc.sync.dma_start(out=whh1[:], in_=d_whh1[:])
            whpT = cpool.tile([H, 3], F32R)
            nc.sync.dma_start(out=whpT[:], in_=d_whp[:])
            b0 = cpool.tile([H, H], F32R)
            nc.sync.dma_start(out=b0[:], in_=d_b0[:])
            b1 = cpool.tile([H, H], F32R)
            nc.sync.dma_start(out=b1[:], in_=d_b1[:])
            ones = cpool.tile([H, TN], F32R)
            nc.sync.dma_start(out=ones[:], in_=d_ones[:])

            # --- state (persistent, updated in place) ---
            h_0 = spool.tile([H, BL], F32R)
            nc.sync.dma_start(out=h_0[:], in_=d_h0[:])
            h_1 = spool.tile([H, BL], F32R)
            nc.sync.dma_start(out=h_1[:], in_=d_h1[:])
            c_0 = spool.tile([H, BL], F32)
            nc.sync.dma_start(out=c_0[:], in_=d_c0[:])
            c_1 = spool.tile([H, BL], F32)
            nc.sync.dma_start(out=c_1[:], in_=d_c1[:])

            def emit_step(t):
                for li in (0, 1):
                    if li == 0:
                        bias, win, rhs_in = b0, weff, h_1
                        wrec, c_st, h_st = whh0, c_0, h_0
                    else:
                        bias, win, rhs_in = b1, wih1, h_0
                        wrec, c_st, h_st = whh1, c_1, h_1
                    for nt in range(NT):
                        sl = slice(nt * TN, (nt + 1) * TN)
                        # gate pre-activations: [i|f|o|2g] in one 4-bank tile
                        quad = qpool.tile([H, 4 * TN], F32, name="quad",
                                          tag="quad")

                        def bank(j):
                            return quad[:, j * TN:(j + 1) * TN]

                        for j in range(4):  # bias (rank-1, packed row-tiles)
                            nc.tensor.matmul(
                                bank(j),
                                bias[32 * j:32 * j + 1, :],
                                ones[32 * j:32 * j + 1, :],
                                start=True, stop=False,
                                tile_position=(32 * j, 0),
                            )
                        for j in range(4):  # input-side
                            js = slice(j * H, (j + 1) * H)
                            nc.tensor.matmul(bank(j), win[:, js],
                                             rhs_in[:, sl],
                                             start=False, stop=False)
                        for j in range(4):  # recurrent-side
                            js = slice(j * H, (j + 1) * H)
                            nc.tensor.matmul(bank(j), wrec[:, js],
                                             h_st[:, sl],
                                             start=False, stop=True)

                        # ONE sigmoid over all four gate chunks
                        sfog = wpool.tile([H, 4 * TN], F32, name="sfog",
                                          tag="sfog")
                        nc.scalar.activation(sfog[:], quad[:], SIG)
                        # cell math: v = sf*c on GPSIMD; fused STT on DVE
                        v = wpool.tile([H, TN], F32, name="v", tag="v")
                        nc.gpsimd.tensor_mul(v[:], sfog[:, TN:2 * TN],
                                             c_st[:, sl])
                        u2 = wpool.tile([H, TN], F32, name="u2", tag="u2")
                        nc.vector.scalar_tensor_tensor(
                            u2[:], sfog[:, 3 * TN:4 * TN], 0.5,
                            sfog[:, 0:TN],
                            mybir.AluOpType.subtract, mybir.AluOpType.mult)
                        nc.vector.scalar_tensor_tensor(
                            c_st[:, sl], u2[:], 2.0, v[:],
                            mybir.AluOpType.mult, mybir.AluOpType.add)
                        tch = wpool.tile([H, TN], F32, name="tch", tag="tch")
                        nc.scalar.activation(tch[:], c_st[:, sl], TANH)
                        nc.vector.tensor_mul(h_st[:, sl],
                                             sfog[:, 2 * TN:3 * TN], tch[:])

                # projection: po[3, TN] = W_hp @ h1 tile (fixed stationary whpT)
                out_sb = opool_sb.tile([3, BL], F32, name="out_sb",
                                       tag="out_sb")
                for nt in range(NT):
                    sl = slice(nt * TN, (nt + 1) * TN)
                    po = qpool.tile([3, TN], F32, name="po", tag="quad")
                    nc.tensor.matmul(po[:], whpT[:], h_1[:, sl],
                                     start=True, stop=True)
                    nc.vector.tensor_copy(out_sb[:, sl], po[:])
                nc.sync.dma_start(out=d_out[t], in_=out_sb[:])

            if loop:
                with tc.For_i(0, reps, 1,
                              hint_engines=(mybir.EngineType.PE,)):
                    for t in range(DSTEPS):
                        emit_step(t)
            else:
                for t in range(DSTEPS):
                    emit_step(t)

    nc.compile()
    return nc


def _get_compiled(reps=1, loop=False):
    key = (reps, loop)
    if key not in _COMPILED:
        _COMPILED[key] = build_bass(reps=reps, loop=loop)
    return _COMPILED[key]


# ---------------------------------------------------------------------------
# Cached-jit SPMD runner (no donation; inputs stay device-resident)
# ---------------------------------------------------------------------------

class _Runner:
    def __init__(self, nc, n_cores):
        import jax
        from jax.experimental.shard_map import shard_map
        from jax.sharding import Mesh, PartitionSpec
        from concourse import bass2jax

        bass2jax.install_neuronx_cc_hook()
        self.jax = jax
        self.n_cores = n_cores
        partition_name = (
            nc.partition_id_tensor.name if nc.partition_id_tensor else None
        )

        in_names, out_names, out_avals, zero_outs = [], [], [], []
        for alloc in nc.m.functions[0].allocations:
            if not isinstance(alloc, mybir.MemoryLocationSet):
                continue
            name = alloc.memorylocations[0].name
            if alloc.kind == "ExternalInput":
                if name != partition_name:
                    in_names.append(name)
            elif alloc.kind == "ExternalOutput":
                out_names.append(name)
                shape = tuple(alloc.tensor_shape)
                dtype = mybir.dt.np(alloc.dtype)
                out_avals.append(jax.core.ShapedArray(shape, dtype))
                zero_outs.append(np.zeros(shape, dtype))

        n_params = len(in_names)
        all_in_names = list(in_names) + list(out_names)
        if partition_name is not None:
            all_in_names.append(partition_name)

        self.in_names = in_names
        self.out_names = out_names
        self.out_avals = out_avals
        self.zero_outs = zero_outs

        def _body(*args):
            operands = list(args)
            if partition_name is not None:
                operands.append(bass2jax.partition_id_tensor())
            outs = bass2jax._bass_exec_p.bind(
                *operands,
                out_avals=tuple(out_avals),
                in_names=tuple(all_in_names),
                out_names=tuple(out_names),
                lowering_input_output_aliases=(),
                sim_require_finite=True,
                sim_require_nnan=True,
                nc=nc,
            )
            return tuple(outs)

        devices = jax.devices()[:n_cores]
        mesh = Mesh(np.asarray(devices), ("core",))
        n_outs = len(out_avals)
        in_specs = (PartitionSpec("core"),) * (n_params + n_outs)
        out_specs = (PartitionSpec("core"),) * n_outs
        self.fn = jax.jit(
            shard_map(_body, mesh=mesh, in_specs=in_specs,
                      out_specs=out_specs, check_rep=False),
            keep_unused=True,
        )
        self.sharding = jax.sharding.NamedSharding(mesh, PartitionSpec("core"))

    def put_inputs(self, in_maps):
        """Ship per-core inputs (concat on axis 0) + zero outputs to devices."""
        arrs = [
            np.concatenate([np.asarray(m[name]) for m in in_maps], axis=0)
            for name in self.in_names
        ]
        arrs += [
            np.zeros((self.n_cores * z.shape[0], *z.shape[1:]), z.dtype)
            for z in self.zero_outs
        ]
        return [self.jax.device_put(a, self.sharding) for a in arrs]

    def launch(self, dev_args):
        return self.fn(*dev_args)

    def fetch(self, out_arrs):
        res = []
        for c in range(self.n_cores):
            res.append({
                name: np.asarray(out_arrs[i]).reshape(
                    self.n_cores, *self.out_avals[i].shape)[c]
                for i, name in enumerate(self.out_names)
            })
        return res

    def run(self, in_maps, tries=5):
        import time as _time
        dev = self.put_inputs(in_maps)
        last = None
        for a in range(tries):
            try:
                outs = self.launch(dev)
                self.jax.block_until_ready(outs)
                return self.fetch(outs)
            except Exception as e:  # transient tunnel/terminal failures
                last = e
                _time.sleep(2.0)
        raise last


def get_runner(reps=1, loop=False):
    key = (reps, loop)
    if key not in _RUNNERS:
        _RUNNERS[key] = _Runner(_get_compiled(reps=reps, loop=loop), N_CORES)
    return _RUNNERS[key]


# ---------------------------------------------------------------------------
# Host-side prep: permutations, step 0, gather
# ---------------------------------------------------------------------------

PERM = (0, 1, 3, 2)  # PyTorch gate order (i,f,g,o) -> kernel order (i,f,o,g)


def _permg(w):
    """Permute gate-chunk rows of a [512, ...] weight (or [512] bias) to
    kernel order (i,f,o,g) and DOUBLE the g chunk (tanh-via-sigmoid)."""
    wp = w.reshape(4, H, *w.shape[1:])[list(PERM)].copy()
    wp[3] *= 2.0
    return wp.reshape(w.shape)


def _bias_rows(b):
    """[512] bias -> [128,128] tile with chunk j (kernel order) at row 32j."""
    out = np.zeros((H, H), np.float32)
    bp = _permg(b)
    for j in range(4):
        out[32 * j] = bp[j * H:(j + 1) * H]
    return out


def _sigmoid(x):
    return 1.0 / (1.0 + np.exp(-x))


def _host_step0(obs_traj_rel, h0, c0, W_ih0, W_hh0, b_ih0, b_hh0,
                W_ih1, W_hh1, b_ih1, b_hh1, W_hp, b_hp):
    """Run reference step 0 in numpy fp32; return post-step state + out0."""
    f = np.float32
    x = obs_traj_rel[-1].astype(f)                      # [B, 3]
    hs = [h0[0].astype(f), h0[1].astype(f)]
    cs = [c0[0].astype(f), c0[1].astype(f)]
    params = ((W_ih0, W_hh0, b_ih0, b_hh0), (W_ih1, W_hh1, b_ih1, b_hh1))
    inp = x
    for l, (Wi, Wh, bi, bh) in enumerate(params):
        gates = inp @ Wi.T.astype(f) + hs[l] @ Wh.T.astype(f) \
            + bi.astype(f) + bh.astype(f)
        i, fg, g, o = np.split(gates, 4, axis=-1)
        c_new = _sigmoid(fg) * cs[l] + _sigmoid(i) * np.tanh(g)
        h_new = _sigmoid(o) * np.tanh(c_new)
        hs[l], cs[l] = h_new, c_new
        inp = h_new
    out0 = inp @ W_hp.T.astype(f) + b_hp.astype(f)      # [B, 3]
    return hs, cs, out0


def prep_inputs(obs_traj_rel, h0, c0, W_ih0, W_hh0, b_ih0, b_hh0,
                W_ih1, W_hh1, b_ih1, b_hh1, W_hp, b_hp):
    f = np.float32
    asc = np.ascontiguousarray

    hs, cs, out0 = _host_step0(
        obs_traj_rel, h0, c0, W_ih0, W_hh0, b_ih0, b_hh0,
        W_ih1, W_hh1, b_ih1, b_hh1, W_hp, b_hp)

    W_eff = (W_ih0.astype(f) @ W_hp.astype(f)).astype(f)       # [512, 128]
    b0_eff = (b_ih0 + b_hh0 + W_ih0 @ b_hp).astype(f)
    b1v = (b_ih1 + b_hh1).astype(f)

    shared = {
        "weff": asc(_permg(W_eff).T),             # [128, 512]
        "whh0": asc(_permg(W_hh0.astype(f)).T),
        "wih1": asc(_permg(W_ih1.astype(f)).T),
        "whh1": asc(_permg(W_hh1.astype(f)).T),
        "whpT": asc(W_hp.T.astype(f)),            # [128, 3]
        "b0": _bias_rows(b0_eff),
        "b1": _bias_rows(b1v),
        "ones": np.ones((H, TN), f),
    }

    h0T = hs[0].T       # [128, 8192]
    h1T = hs[1].T
    c0T = cs[0].T
    c1T = cs[1].T

    in_maps = []
    for c in range(N_CORES):
        bsl = slice(c * BL, (c + 1) * BL)
        m = dict(shared)
        m["h0T"] = asc(h0T[:, bsl].astype(f))
        m["h1T"] = asc(h1T[:, bsl].astype(f))
        m["c0T"] = asc(c0T[:, bsl].astype(f))
        m["c1T"] = asc(c1T[:, bsl].astype(f))
        in_maps.append(m)
    return in_maps, out0


def gather(res_results, out0, b_hp):
    # per-core preds [29, 3, 1024]; global step t = device step t-1
    dev = np.concatenate([res_results[c]["preds"] for c in range(N_CORES)],
                         axis=2)                       # [29, 3, 8192]
    preds = np.empty((PRED, B, 3), np.float32)
    preds[0] = out0
    preds[1:] = dev.transpose(0, 2, 1) + b_hp.astype(np.float32)
    return preds


def kernel(**inputs):
    inputs = {k: np.asarray(v) for k, v in inputs.items()}
    in_maps, out0 = prep_inputs(**inputs)
    r = get_runner(reps=1, loop=False)
    res = r.run(in_maps)
    return gather(res, out0, inputs["b_hp"])


# revision 16
# speedup vs baseline: 1.1583x; 1.0742x over previous
"""Trainium2 Bass kernel for a 2-layer LSTM decoder (nn_Decoder).

Reference computation (per step, 30 steps):
    L0: gates = W_ih0 @ x + W_hh0 @ h0 + b;  LSTM cell -> h0', c0'
    L1: gates = W_ih1 @ h0' + W_hh1 @ h1 + b; LSTM cell -> h1', c1'
    out = W_hp @ h1' + b_hp ;  x_next = out

Structure:
 1. Step 0 (the only step that consumes the observed x) runs on the HOST in
    numpy; the device receives the post-step-0 state and runs 29 UNIFORM
    steps.  For t >= 1, x_t = W_hp @ h1_{t-1} + b_hp folds into an effective
    weight on h1_{t-1}: W_eff = W_ih0 @ W_hp, b0_eff = b_ih0 + b_hh0 +
    W_ih0 @ b_hp.  This removes the K=3 matmul entirely (lhsT/rhs with
    partition-dim 3 wedges the exec unit: NRT_EXEC_UNIT_UNRECOVERABLE).
 2. Gate biases are injected into PSUM by packed K=1 rank-1 matmuls
    (bias-row x ones) at tile_position=(32j,0) - the 4 row-group matmuls run
    concurrently in the PE array, so the whole bias injection costs ~one
    matmul slot.
 3. Gate chunk order is host-permuted from PyTorch's (i,f,g,o) to (i,f,o,g);
    the g-chunk weights/bias are DOUBLED on the host, so tanh(g) =
    2*sigmoid(2g) - 1 and a single sigmoid over the whole [i|f|o|2g] 4-bank
    PSUM tile covers all four gates.  The cell math then runs as fused
    scalar_tensor_tensor ops:
        u2 = (sg - 0.5) * si            ( = si * tanh(g) / 2 )
        c' = (u2 * 2) + (sf * c)
    with the (sf * c) multiply offloaded to the otherwise-idle GPSIMD.
 4. The projection W_hp @ h1 runs with W_hp^T as the STATIONARY operand
    (fixed weights, 3 columns -> negligible LDWEIGHTS), output [3, 512] per
    batch tile; b_hp is added on the host after gather.

Layout: state kept on-chip as [H=128 partitions, B_local free]; batch
data-parallel over 8 cores (1024 each), 2 tiles of 512 per core.
Matmuls run as float32r (fp32 data, full PE rate at N >= 256).
"""

import numpy as np

import concourse.bass as bass
import concourse.mybir as mybir
from concourse import bacc
from concourse.tile import TileContext

N_CORES = 8
B = 8192
BL = B // N_CORES  # 1024 per core
H = 128
PRED = 30
DSTEPS = PRED - 1   # 29 device steps; step 0 on host
NT = 2              # batch tiles per core
TN = BL // NT       # 512 = one PSUM bank of fp32

F32 = mybir.dt.float32
F32R = mybir.dt.float32r

_COMPILED = {}
_RUNNERS = {}


OPTS = {
    "v_eng": "gpsimd",     # engine for v = sf*c
    "h_eng": "gpsimd",     # engine for h = so*tanh(c)
    "u2_eng": "dve",       # engine for u2 = (sg-0.5)*si
    "interleave": True,    # interleave T0/T1 emission within a layer
    "wbufs": 3,
    "split_sig": False,    # chunk order (i,f,g,o): sigma over [i|f|2g], then [o]
    "per_tile_eng": False,  # T0 cell-math on DVE, T1 on GPSIMD
    "obank_last": False,    # emit o-bank matmuls after the i/f/g banks
    "proj_delay": True,     # emit projection of step t after step t+1's gates
    "wrec_first": True,     # recurrent-side MMs before input-side (h-chain last)
    "sig3": False,          # sigma as [i|f], [2g], [o] (3 ops)
}


def build_bass(reps=1, loop=False):
    """29-step uniform LSTM decoder body.  loop=True wraps the body in a
    hardware For_i(0, reps) loop (same NEFF size for any reps) - used for
    timing; the graded path uses loop=False (straight-line, reps=1)."""
    nc = bacc.Bacc("TRN2", target_bir_lowering=False, debug=False)

    # Per-core state after host step 0
    d_h0 = nc.declare_dram_parameter("h0T", [H, BL], F32R, isOutput=False)
    d_h1 = nc.declare_dram_parameter("h1T", [H, BL], F32R, isOutput=False)
    d_c0 = nc.declare_dram_parameter("c0T", [H, BL], F32, isOutput=False)
    d_c1 = nc.declare_dram_parameter("c1T", [H, BL], F32, isOutput=False)
    # Replicated weights (gate chunk order i,f,o,g)
    d_weff = nc.declare_dram_parameter("weff", [H, 4 * H], F32R, isOutput=False)
    d_whh0 = nc.declare_dram_parameter("whh0", [H, 4 * H], F32R, isOutput=False)
    d_wih1 = nc.declare_dram_parameter("wih1", [H, 4 * H], F32R, isOutput=False)
    d_whh1 = nc.declare_dram_parameter("whh1", [H, 4 * H], F32R, isOutput=False)
    d_whp = nc.declare_dram_parameter("whpT", [H, 3], F32R, isOutput=False)
    # Bias rows: [128,128], row 32*j holds bias chunk j (order i,f,o,g)
    d_b0 = nc.declare_dram_parameter("b0", [H, H], F32R, isOutput=False)
    d_b1 = nc.declare_dram_parameter("b1", [H, H], F32R, isOutput=False)
    d_ones = nc.declare_dram_parameter("ones", [H, TN], F32R, isOutput=False)
    # Output: step t (device) = global step t+1; [3, BL] = (W_hp @ h1)^  (no bias)
    d_out = nc.declare_dram_parameter("preds", [DSTEPS, 3, BL], F32,
                                      isOutput=True)

    SIG = mybir.ActivationFunctionType.Sigmoid
    TANH = mybir.ActivationFunctionType.Tanh

    with TileContext(nc) as tc:
        with (
            tc.tile_pool(name="const", bufs=1) as cpool,
            tc.tile_pool(name="state", bufs=1) as spool,
            tc.tile_pool(name="work", bufs=OPTS["wbufs"]) as wpool,
            tc.tile_pool(name="outp", bufs=2) as opool_sb,
            tc.tile_pool(name="pq_pool", bufs=2, space="PSUM") as qpool,
        ):
            # --- load constants / weights ---
            weff = cpool.tile([H, 4 * H], F32R)
            nc.sync.dma_start(out=weff[:], in_=d_weff[:])
            whh0 = cpool.tile([H, 4 * H], F32R)
            nc.sync.dma_start(out=whh0[:], in_=d_whh0[:])
            wih1 = cpool.tile([H, 4 * H], F32R)
            nc.sync.dma_start(out=wih1[:], in_=d_wih1[:])
            whh1 = cpool.tile([H, 4 * H], F32R)
            nc.sync.dma_start(out=whh1[:], in_=d_whh1[:])
            whpT = cpool.tile([H, 3], F32R)
            nc.sync.dma_start(out=whpT[:], in_=d_whp[:])
            b0 = cpool.tile([H, H], F32R)
            nc.sync.dma_start(out=b0[:], in_=d_b0[:])
            b1 = cpool.tile([H, H], F32R)
            nc.sync.dma_start(out=b1[:], in_=d_b1[:])
            ones = cpool.tile([H, TN], F32R)
            nc.sync.dma_start(out=ones[:], in_=d_ones[:])

            # --- state (persistent, updated in place) ---
            h_0 = spool.tile([H, BL], F32R)
            nc.sync.dma_start(out=h_0[:], in_=d_h0[:])
            h_1 = spool.tile([H, BL], F32R)
            nc.sync.dma_start(out=h_1[:], in_=d_h1[:])
            c_0 = spool.tile([H, BL], F32)
            nc.sync.dma_start(out=c_0[:], in_=d_c0[:])
            c_1 = spool.tile([H, BL], F32)
            nc.sync.dma_start(out=c_1[:], in_=d_c1[:])

            def eng(which):
                return nc.gpsimd if which == "gpsimd" else nc.vector

            def emit_gates(bias, win, rhs_in, wrec, h_st, sl):
                quad = qpool.tile([H, 4 * TN], F32, name="quad", tag="quad")

                def bank(j):
                    return quad[:, j * TN:(j + 1) * TN]

                if OPTS["obank_last"] and OPTS["split_sig"]:
                    jorder = (0, 1, 2, 3)   # (i,f,g) critical first, then o
                    phased = True
                else:
                    jorder = (0, 1, 2, 3)
                    phased = False
                def mm_bias(j, start=True):
                    nc.tensor.matmul(
                        bank(j), bias[32 * j:32 * j + 1, :],
                        ones[32 * j:32 * j + 1, :],
                        start=start, stop=False, tile_position=(32 * j, 0))

                def mm_in(j, stop=False):
                    js = slice(j * H, (j + 1) * H)
                    nc.tensor.matmul(bank(j), win[:, js], rhs_in[:, sl],
                                     start=False, stop=stop)

                def mm_rec(j, stop=True):
                    js = slice(j * H, (j + 1) * H)
                    nc.tensor.matmul(bank(j), wrec[:, js], h_st[:, sl],
                                     start=False, stop=stop)

                groups = [(0, 1, 2), (3,)] if phased else [(0, 1, 2, 3)]
                for grp in groups:
                    for j in grp:
                        mm_bias(j)
                    if OPTS["wrec_first"]:
                        for j in grp:
                            mm_rec(j, stop=False)
                        for j in grp:
                            mm_in(j, stop=True)
                    else:
                        for j in grp:
                            mm_in(j)
                        for j in grp:
                            mm_rec(j)
                return quad

            def emit_sig(quad):
                sfog = wpool.tile([H, 4 * TN], F32, name="sfog", tag="sfog")
                if OPTS["split_sig"] and OPTS["sig3"]:
                    nc.scalar.activation(sfog[:, 0:2 * TN], quad[:, 0:2 * TN],
                                         SIG)
                    nc.scalar.activation(sfog[:, 2 * TN:3 * TN],
                                         quad[:, 2 * TN:3 * TN], SIG)
                    nc.scalar.activation(sfog[:, 3 * TN:4 * TN],
                                         quad[:, 3 * TN:4 * TN], SIG)
                elif OPTS["split_sig"]:
                    nc.scalar.activation(sfog[:, 0:3 * TN], quad[:, 0:3 * TN],
                                         SIG)
                    nc.scalar.activation(sfog[:, 3 * TN:4 * TN],
                                         quad[:, 3 * TN:4 * TN], SIG)
                else:
                    nc.scalar.activation(sfog[:], quad[:], SIG)
                return sfog

            def chunk(sfog, name):
                order = "ifgo" if OPTS["split_sig"] else "ifog"
                j = order.index(name)
                return sfog[:, j * TN:(j + 1) * TN]

            def _eng_for(kind, nt):
                if OPTS["per_tile_eng"]:
                    return eng("dve" if nt == 0 else "gpsimd")
                return eng(OPTS[kind])

            def emit_cell_pre(sfog, c_st, sl, nt=0):
                v = wpool.tile([H, TN], F32, name="v", tag="v")
                _eng_for("v_eng", nt).tensor_mul(v[:], chunk(sfog, "f"),
                                                 c_st[:, sl])
                u2 = wpool.tile([H, TN], F32, name="u2", tag="u2")
                eng(OPTS["u2_eng"]).scalar_tensor_tensor(
                    u2[:], chunk(sfog, "g"), 0.5, chunk(sfog, "i"),
                    mybir.AluOpType.subtract, mybir.AluOpType.mult)
                nc.vector.scalar_tensor_tensor(
                    c_st[:, sl], u2[:], 2.0, v[:],
                    mybir.AluOpType.mult, mybir.AluOpType.add)

            def emit_cell_post(sfog, c_st, h_st, sl, nt=0):
                tch = wpool.tile([H, TN], F32, name="tch", tag="tch")
                nc.scalar.activation(tch[:], c_st[:, sl], TANH)
                _eng_for("h_eng", nt).tensor_mul(
                    h_st[:, sl], chunk(sfog, "o"), tch[:])

            def emit_layer(li):
                if li == 0:
                    bias, win, rhs_in = b0, weff, h_1
                    wrec, c_st, h_st = whh0, c_0, h_0
                else:
                    bias, win, rhs_in = b1, wih1, h_0
                    wrec, c_st, h_st = whh1, c_1, h_1
                sls = [slice(nt * TN, (nt + 1) * TN) for nt in range(NT)]
                if OPTS["interleave"]:
                    quads = [emit_gates(bias, win, rhs_in, wrec, h_st, sl)
                             for sl in sls]
                    sfogs = [emit_sig(q) for q in quads]
                    for nt, (sfog, sl) in enumerate(zip(sfogs, sls)):
                        emit_cell_pre(sfog, c_st, sl, nt)
                    for nt, (sfog, sl) in enumerate(zip(sfogs, sls)):
                        emit_cell_post(sfog, c_st, h_st, sl, nt)
                else:
                    for nt, sl in enumerate(sls):
                        quad = emit_gates(bias, win, rhs_in, wrec, h_st, sl)
                        sfog = emit_sig(quad)
                        emit_cell_pre(sfog, c_st, sl, nt)
                        emit_cell_post(sfog, c_st, h_st, sl, nt)

            def emit_step(t):
                emit_layer(0)
                if OPTS["proj_delay"] and t > 0:
                    emit_proj(t - 1)
                emit_layer(1)
                if not OPTS["proj_delay"]:
                    emit_proj(t)

            def emit_proj(t):
                # projection: po[3, TN] = W_hp @ h1 tile (stationary whpT)
                out_sb = opool_sb.tile([3, BL], F32, name="out_sb",
                                       tag="out_sb")
                for nt in range(NT):
                    sl = slice(nt * TN, (nt + 1) * TN)
                    po = qpool.tile([3, TN], F32, name="po", tag="quad")
                    nc.tensor.matmul(po[:], whpT[:], h_1[:, sl],
                                     start=True, stop=True)
                    nc.vector.tensor_copy(out_sb[:, sl], po[:])
                nc.sync.dma_start(out=d_out[t], in_=out_sb[:])

            if loop:
                with tc.For_i(0, reps, 1,
                              hint_engines=(mybir.EngineType.PE,)):
                    for t in range(DSTEPS):
                        emit_step(t)
                    if OPTS["proj_delay"]:
                        emit_proj(DSTEPS - 1)
            else:
                for t in range(DSTEPS):
                    emit_step(t)
                if OPTS["proj_delay"]:
                    emit_proj(DSTEPS - 1)

    nc.compile()
    return nc


def _get_compiled(reps=1, loop=False):
    key = (reps, loop)
    if key not in _COMPILED:
        _COMPILED[key] = build_bass(reps=reps, loop=loop)
    return _COMPILED[key]


# ---------------------------------------------------------------------------
# Cached-jit SPMD runner (no donation; inputs stay device-resident)
# ---------------------------------------------------------------------------

class _Runner:
    def __init__(self, nc, n_cores):
        import jax
        from jax.experimental.shard_map import shard_map
        from jax.sharding import Mesh, PartitionSpec
        from concourse import bass2jax

        bass2jax.install_neuronx_cc_hook()
        self.jax = jax
        self.n_cores = n_cores
        partition_name = (
            nc.partition_id_tensor.name if nc.partition_id_tensor else None
        )

        in_names, out_names, out_avals, zero_outs = [], [], [], []
        for alloc in nc.m.functions[0].allocations:
            if not isinstance(alloc, mybir.MemoryLocationSet):
                continue
            name = alloc.memorylocations[0].name
            if alloc.kind == "ExternalInput":
                if name != partition_name:
                    in_names.append(name)
            elif alloc.kind == "ExternalOutput":
                out_names.append(name)
                shape = tuple(alloc.tensor_shape)
                dtype = mybir.dt.np(alloc.dtype)
                out_avals.append(jax.core.ShapedArray(shape, dtype))
                zero_outs.append(np.zeros(shape, dtype))

        n_params = len(in_names)
        all_in_names = list(in_names) + list(out_names)
        if partition_name is not None:
            all_in_names.append(partition_name)

        self.in_names = in_names
        self.out_names = out_names
        self.out_avals = out_avals
        self.zero_outs = zero_outs

        def _body(*args):
            operands = list(args)
            if partition_name is not None:
                operands.append(bass2jax.partition_id_tensor())
            outs = bass2jax._bass_exec_p.bind(
                *operands,
                out_avals=tuple(out_avals),
                in_names=tuple(all_in_names),
                out_names=tuple(out_names),
                lowering_input_output_aliases=(),
                sim_require_finite=True,
                sim_require_nnan=True,
                nc=nc,
            )
            return tuple(outs)

        devices = jax.devices()[:n_cores]
        mesh = Mesh(np.asarray(devices), ("core",))
        n_outs = len(out_avals)
        in_specs = (PartitionSpec("core"),) * (n_params + n_outs)
        out_specs = (PartitionSpec("core"),) * n_outs
        self.fn = jax.jit(
            shard_map(_body, mesh=mesh, in_specs=in_specs,
                      out_specs=out_specs, check_rep=False),
            keep_unused=True,
        )
        self.sharding = jax.sharding.NamedSharding(mesh, PartitionSpec("core"))

    def put_inputs(self, in_maps):
        """Ship per-core inputs (concat on axis 0) + zero outputs to devices."""
        arrs = [
            np.concatenate([np.asarray(m[name]) for m in in_maps], axis=0)
            for name in self.in_names
        ]
        arrs += [
            np.zeros((self.n_cores * z.shape[0], *z.shape[1:]), z.dtype)
            for z in self.zero_outs
        ]
        return [self.jax.device_put(a, self.sharding) for a in arrs]

    def launch(self, dev_args):
        return self.fn(*dev_args)

    def fetch(self, out_arrs):
        res = []
        for c in range(self.n_cores):
            res.append({
                name: np.asarray(out_arrs[i]).reshape(
                    self.n_cores, *self.out_avals[i].shape)[c]
                for i, name in enumerate(self.out_names)
            })
        return res

    def run(self, in_maps, tries=5):
        import time as _time
        dev = self.put_inputs(in_maps)
        last = None
        for a in range(tries):
            try:
                outs = self.launch(dev)
                self.jax.block_until_ready(outs)
                return self.fetch(outs)
            except Exception as e:  # transient tunnel/terminal failures
                last = e
                _time.sleep(2.0)
        raise last


def get_runner(reps=1, loop=False):
    key = (reps, loop)
    if key not in _RUNNERS:
        _RUNNERS[key] = _Runner(_get_compiled(reps=reps, loop=loop), N_CORES)
    return _RUNNERS[key]


# ---------------------------------------------------------------------------
# Host-side prep: permutations, step 0, gather
# ---------------------------------------------------------------------------

def _permg(w):
    """Permute gate-chunk rows of a [512, ...] weight (or [512] bias) to
    kernel chunk order and DOUBLE the g chunk (tanh-via-sigmoid)."""
    if OPTS["split_sig"]:
        perm, gpos = (0, 1, 2, 3), 2      # (i,f,g,o)
    else:
        perm, gpos = (0, 1, 3, 2), 3      # (i,f,o,g)
    wp = w.reshape(4, H, *w.shape[1:])[list(perm)].copy()
    wp[gpos] *= 2.0
    return wp.reshape(w.shape)


def _bias_rows(b):
    """[512] bias -> [128,128] tile with chunk j (kernel order) at row 32j."""
    out = np.zeros((H, H), np.float32)
    bp = _permg(b)
    for j in range(4):
        out[32 * j] = bp[j * H:(j + 1) * H]
    return out


def _sigmoid(x):
    return 1.0 / (1.0 + np.exp(-x))


def _host_step0(obs_traj_rel, h0, c0, W_ih0, W_hh0, b_ih0, b_hh0,
                W_ih1, W_hh1, b_ih1, b_hh1, W_hp, b_hp):
    """Run reference step 0 in numpy fp32; return post-step state + out0."""
    f = np.float32
    x = obs_traj_rel[-1].astype(f)                      # [B, 3]
    hs = [h0[0].astype(f), h0[1].astype(f)]
    cs = [c0[0].astype(f), c0[1].astype(f)]
    params = ((W_ih0, W_hh0, b_ih0, b_hh0), (W_ih1, W_hh1, b_ih1, b_hh1))
    inp = x
    for l, (Wi, Wh, bi, bh) in enumerate(params):
        gates = inp @ Wi.T.astype(f) + hs[l] @ Wh.T.astype(f) \
            + bi.astype(f) + bh.astype(f)
        i, fg, g, o = np.split(gates, 4, axis=-1)
        c_new = _sigmoid(fg) * cs[l] + _sigmoid(i) * np.tanh(g)
        h_new = _sigmoid(o) * np.tanh(c_new)
        hs[l], cs[l] = h_new, c_new
        inp = h_new
    out0 = inp @ W_hp.T.astype(f) + b_hp.astype(f)      # [B, 3]
    return hs, cs, out0


def prep_inputs(obs_traj_rel, h0, c0, W_ih0, W_hh0, b_ih0, b_hh0,
                W_ih1, W_hh1, b_ih1, b_hh1, W_hp, b_hp):
    f = np.float32
    asc = np.ascontiguousarray

    hs, cs, out0 = _host_step0(
        obs_traj_rel, h0, c0, W_ih0, W_hh0, b_ih0, b_hh0,
        W_ih1, W_hh1, b_ih1, b_hh1, W_hp, b_hp)

    W_eff = (W_ih0.astype(f) @ W_hp.astype(f)).astype(f)       # [512, 128]
    b0_eff = (b_ih0 + b_hh0 + W_ih0 @ b_hp).astype(f)
    b1v = (b_ih1 + b_hh1).astype(f)

    shared = {
        "weff": asc(_permg(W_eff).T),             # [128, 512]
        "whh0": asc(_permg(W_hh0.astype(f)).T),
        "wih1": asc(_permg(W_ih1.astype(f)).T),
        "whh1": asc(_permg(W_hh1.astype(f)).T),
        "whpT": asc(W_hp.T.astype(f)),            # [128, 3]
        "b0": _bias_rows(b0_eff),
        "b1": _bias_rows(b1v),
        "ones": np.ones((H, TN), f),
    }

    h0T = hs[0].T       # [128, 8192]
    h1T = hs[1].T
    c0T = cs[0].T
    c1T = cs[1].T

    in_maps = []
    for c in range(N_CORES):
        bsl = slice(c * BL, (c + 1) * BL)
        m = dict(shared)
        m["h0T"] = asc(h0T[:, bsl].astype(f))
        m["h1T"] = asc(h1T[:, bsl].astype(f))
        m["c0T"] = asc(c0T[:, bsl].astype(f))
        m["c1T"] = asc(c1T[:, bsl].astype(f))
        in_maps.append(m)
    return in_maps, out0


def gather(res_results, out0, b_hp):
    # per-core preds [29, 3, 1024]; global step t = device step t-1
    dev = np.concatenate([res_results[c]["preds"] for c in range(N_CORES)],
                         axis=2)                       # [29, 3, 8192]
    preds = np.empty((PRED, B, 3), np.float32)
    preds[0] = out0
    preds[1:] = dev.transpose(0, 2, 1) + b_hp.astype(np.float32)
    return preds


def kernel(**inputs):
    inputs = {k: np.asarray(v) for k, v in inputs.items()}
    in_maps, out0 = prep_inputs(**inputs)
    r = get_runner(reps=1, loop=False)
    res = r.run(in_maps)
    return gather(res, out0, inputs["b_hp"])


# revision 17
# speedup vs baseline: 1.7717x; 1.5295x over previous
"""Trainium2 Bass kernel for a 2-layer LSTM decoder (nn_Decoder).

Reference computation (per step, 30 steps):
    L0: gates = W_ih0 @ x + W_hh0 @ h0 + b;  LSTM cell -> h0', c0'
    L1: gates = W_ih1 @ h0' + W_hh1 @ h1 + b; LSTM cell -> h1', c1'
    out = W_hp @ h1' + b_hp ;  x_next = out

Structure:
 1. Step 0 (the only step that consumes the observed x) runs on the HOST in
    numpy; the device receives the post-step-0 state and runs 29 UNIFORM
    steps.  For t >= 1, x_t = W_hp @ h1_{t-1} + b_hp folds into an effective
    weight on h1_{t-1}: W_eff = W_ih0 @ W_hp, b0_eff = b_ih0 + b_hh0 +
    W_ih0 @ b_hp.  This removes the K=3 matmul entirely (lhsT/rhs with
    partition-dim 3 wedges the exec unit: NRT_EXEC_UNIT_UNRECOVERABLE).
 2. Gate biases are injected into PSUM by packed K=1 rank-1 matmuls
    (bias-row x ones) at tile_position=(32j,0) - the 4 row-group matmuls run
    concurrently in the PE array, so the whole bias injection costs ~one
    matmul slot.
 3. Gate chunk order is host-permuted from PyTorch's (i,f,g,o) to (i,f,o,g);
    the g-chunk weights/bias are DOUBLED on the host, so tanh(g) =
    2*sigmoid(2g) - 1 and a single sigmoid over the whole [i|f|o|2g] 4-bank
    PSUM tile covers all four gates.  The cell math then runs as fused
    scalar_tensor_tensor ops:
        u2 = (sg - 0.5) * si            ( = si * tanh(g) / 2 )
        c' = (u2 * 2) + (sf * c)
    with the (sf * c) multiply offloaded to the otherwise-idle GPSIMD.
 4. The projection W_hp @ h1 runs with W_hp^T as the STATIONARY operand
    (fixed weights, 3 columns -> negligible LDWEIGHTS), output [3, 512] per
    batch tile; b_hp is added on the host after gather.

Layout: state kept on-chip as [H=128 partitions, B_local free]; batch
data-parallel over 8 cores (1024 each), 2 tiles of 512 per core.
Matmuls run as float32r (fp32 data, full PE rate at N >= 256).
"""

import numpy as np

import concourse.bass as bass
import concourse.mybir as mybir
from concourse import bacc
from concourse.tile import TileContext

N_CORES = 8
B = 8192
BL = B // N_CORES  # 1024 per core
H = 128
PRED = 30
DSTEPS = PRED - 1   # 29 device steps; step 0 on host
NT = 2              # batch tiles per core
TN = BL // NT       # 512 = one PSUM bank of fp32

F32 = mybir.dt.float32
F32R = mybir.dt.float32r

_COMPILED = {}
_RUNNERS = {}


OPTS = {
    "v_eng": "dve",        # engine for v = sf*c
    "h_eng": "dve",        # engine for h = so*tanh(c)
    "u2_eng": "dve",       # engine for u2 = (sg-0.5)*si
    "interleave": True,    # interleave T0/T1 emission within a layer
    "wbufs": 3,
    "split_sig": False,    # chunk order (i,f,g,o): sigma over [i|f|2g], then [o]
    "per_tile_eng": False,  # T0 cell-math on DVE, T1 on GPSIMD
    "obank_last": False,    # emit o-bank matmuls after the i/f/g banks
    "proj_delay": True,     # emit projection of step t after step t+1's gates
    "wrec_first": True,     # recurrent-side MMs before input-side (h-chain last)
    "sig3": False,          # sigma as [i|f], [2g], [o] (3 ops)
}


def build_bass(reps=1, loop=False):
    """29-step uniform LSTM decoder body.  loop=True wraps the body in a
    hardware For_i(0, reps) loop (same NEFF size for any reps) - used for
    timing; the graded path uses loop=False (straight-line, reps=1)."""
    nc = bacc.Bacc("TRN2", target_bir_lowering=False, debug=False)

    # Per-core state after host step 0
    d_h0 = nc.declare_dram_parameter("h0T", [H, BL], F32R, isOutput=False)
    d_h1 = nc.declare_dram_parameter("h1T", [H, BL], F32R, isOutput=False)
    d_c0 = nc.declare_dram_parameter("c0T", [H, BL], F32, isOutput=False)
    d_c1 = nc.declare_dram_parameter("c1T", [H, BL], F32, isOutput=False)
    # Replicated weights (gate chunk order i,f,o,g)
    d_weff = nc.declare_dram_parameter("weff", [H, 4 * H], F32R, isOutput=False)
    d_whh0 = nc.declare_dram_parameter("whh0", [H, 4 * H], F32R, isOutput=False)
    d_wih1 = nc.declare_dram_parameter("wih1", [H, 4 * H], F32R, isOutput=False)
    d_whh1 = nc.declare_dram_parameter("whh1", [H, 4 * H], F32R, isOutput=False)
    d_whp = nc.declare_dram_parameter("whpT", [H, 3], F32R, isOutput=False)
    # Bias rows: [128,128], row 32*j holds bias chunk j (order i,f,o,g)
    d_b0 = nc.declare_dram_parameter("b0", [H, H], F32R, isOutput=False)
    d_b1 = nc.declare_dram_parameter("b1", [H, H], F32R, isOutput=False)
    d_ones = nc.declare_dram_parameter("ones", [H, TN], F32R, isOutput=False)
    # Output: step t (device) = global step t+1; [3, BL] = (W_hp @ h1)^  (no bias)
    d_out = nc.declare_dram_parameter("preds", [DSTEPS, 3, BL], F32,
                                      isOutput=True)

    SIG = mybir.ActivationFunctionType.Sigmoid
    TANH = mybir.ActivationFunctionType.Tanh

    with TileContext(nc) as tc:
        with (
            tc.tile_pool(name="const", bufs=1) as cpool,
            tc.tile_pool(name="state", bufs=1) as spool,
            tc.tile_pool(name="work", bufs=OPTS["wbufs"]) as wpool,
            tc.tile_pool(name="outp", bufs=2) as opool_sb,
            tc.tile_pool(name="pq_pool", bufs=2, space="PSUM") as qpool,
        ):
            # --- load constants / weights ---
            weff = cpool.tile([H, 4 * H], F32R)
            nc.sync.dma_start(out=weff[:], in_=d_weff[:])
            whh0 = cpool.tile([H, 4 * H], F32R)
            nc.sync.dma_start(out=whh0[:], in_=d_whh0[:])
            wih1 = cpool.tile([H, 4 * H], F32R)
            nc.sync.dma_start(out=wih1[:], in_=d_wih1[:])
            whh1 = cpool.tile([H, 4 * H], F32R)
            nc.sync.dma_start(out=whh1[:], in_=d_whh1[:])
            whpT = cpool.tile([H, 3], F32R)
            nc.sync.dma_start(out=whpT[:], in_=d_whp[:])
            b0 = cpool.tile([H, H], F32R)
            nc.sync.dma_start(out=b0[:], in_=d_b0[:])
            b1 = cpool.tile([H, H], F32R)
            nc.sync.dma_start(out=b1[:], in_=d_b1[:])
            ones = cpool.tile([H, TN], F32R)
            nc.sync.dma_start(out=ones[:], in_=d_ones[:])

            # --- state (persistent, updated in place) ---
            h_0 = spool.tile([H, BL], F32R)
            nc.sync.dma_start(out=h_0[:], in_=d_h0[:])
            h_1 = spool.tile([H, BL], F32R)
            nc.sync.dma_start(out=h_1[:], in_=d_h1[:])
            c_0 = spool.tile([H, BL], F32)
            nc.sync.dma_start(out=c_0[:], in_=d_c0[:])
            c_1 = spool.tile([H, BL], F32)
            nc.sync.dma_start(out=c_1[:], in_=d_c1[:])

            def eng(which):
                return nc.gpsimd if which == "gpsimd" else nc.vector

            def emit_gates(bias, win, rhs_in, wrec, h_st, sl):
                quad = qpool.tile([H, 4 * TN], F32, name="quad", tag="quad")

                def bank(j):
                    return quad[:, j * TN:(j + 1) * TN]

                if OPTS["obank_last"] and OPTS["split_sig"]:
                    jorder = (0, 1, 2, 3)   # (i,f,g) critical first, then o
                    phased = True
                else:
                    jorder = (0, 1, 2, 3)
                    phased = False
                def mm_bias(j, start=True):
                    nc.tensor.matmul(
                        bank(j), bias[32 * j:32 * j + 1, :],
                        ones[32 * j:32 * j + 1, :],
                        start=start, stop=False, tile_position=(32 * j, 0))

                def mm_in(j, stop=False):
                    js = slice(j * H, (j + 1) * H)
                    nc.tensor.matmul(bank(j), win[:, js], rhs_in[:, sl],
                                     start=False, stop=stop)

                def mm_rec(j, stop=True):
                    js = slice(j * H, (j + 1) * H)
                    nc.tensor.matmul(bank(j), wrec[:, js], h_st[:, sl],
                                     start=False, stop=stop)

                groups = [(0, 1, 2), (3,)] if phased else [(0, 1, 2, 3)]
                for grp in groups:
                    for j in grp:
                        mm_bias(j)
                    if OPTS["wrec_first"]:
                        for j in grp:
                            mm_rec(j, stop=False)
                        for j in grp:
                            mm_in(j, stop=True)
                    else:
                        for j in grp:
                            mm_in(j)
                        for j in grp:
                            mm_rec(j)
                return quad

            def emit_sig(quad):
                sfog = wpool.tile([H, 4 * TN], F32, name="sfog", tag="sfog")
                if OPTS["split_sig"] and OPTS["sig3"]:
                    nc.scalar.activation(sfog[:, 0:2 * TN], quad[:, 0:2 * TN],
                                         SIG)
                    nc.scalar.activation(sfog[:, 2 * TN:3 * TN],
                                         quad[:, 2 * TN:3 * TN], SIG)
                    nc.scalar.activation(sfog[:, 3 * TN:4 * TN],
                                         quad[:, 3 * TN:4 * TN], SIG)
                elif OPTS["split_sig"]:
                    nc.scalar.activation(sfog[:, 0:3 * TN], quad[:, 0:3 * TN],
                                         SIG)
                    nc.scalar.activation(sfog[:, 3 * TN:4 * TN],
                                         quad[:, 3 * TN:4 * TN], SIG)
                else:
                    nc.scalar.activation(sfog[:], quad[:], SIG)
                return sfog

            def chunk(sfog, name):
                order = "ifgo" if OPTS["split_sig"] else "ifog"
                j = order.index(name)
                return sfog[:, j * TN:(j + 1) * TN]

            def _eng_for(kind, nt):
                if OPTS["per_tile_eng"]:
                    return eng("dve" if nt == 0 else "gpsimd")
                return eng(OPTS[kind])

            def emit_cell_pre(sfog, c_st, sl, nt=0):
                v = wpool.tile([H, TN], F32, name="v", tag="v")
                _eng_for("v_eng", nt).tensor_mul(v[:], chunk(sfog, "f"),
                                                 c_st[:, sl])
                u2 = wpool.tile([H, TN], F32, name="u2", tag="u2")
                eng(OPTS["u2_eng"]).scalar_tensor_tensor(
                    u2[:], chunk(sfog, "g"), 0.5, chunk(sfog, "i"),
                    mybir.AluOpType.subtract, mybir.AluOpType.mult)
                nc.vector.scalar_tensor_tensor(
                    c_st[:, sl], u2[:], 2.0, v[:],
                    mybir.AluOpType.mult, mybir.AluOpType.add)

            def emit_cell_post(sfog, c_st, h_st, sl, nt=0):
                tch = wpool.tile([H, TN], F32, name="tch", tag="tch")
                nc.scalar.activation(tch[:], c_st[:, sl], TANH)
                _eng_for("h_eng", nt).tensor_mul(
                    h_st[:, sl], chunk(sfog, "o"), tch[:])

            def emit_layer(li):
                if li == 0:
                    bias, win, rhs_in = b0, weff, h_1
                    wrec, c_st, h_st = whh0, c_0, h_0
                else:
                    bias, win, rhs_in = b1, wih1, h_0
                    wrec, c_st, h_st = whh1, c_1, h_1
                sls = [slice(nt * TN, (nt + 1) * TN) for nt in range(NT)]
                if OPTS["interleave"]:
                    quads = [emit_gates(bias, win, rhs_in, wrec, h_st, sl)
                             for sl in sls]
                    sfogs = [emit_sig(q) for q in quads]
                    for nt, (sfog, sl) in enumerate(zip(sfogs, sls)):
                        emit_cell_pre(sfog, c_st, sl, nt)
                    for nt, (sfog, sl) in enumerate(zip(sfogs, sls)):
                        emit_cell_post(sfog, c_st, h_st, sl, nt)
                else:
                    for nt, sl in enumerate(sls):
                        quad = emit_gates(bias, win, rhs_in, wrec, h_st, sl)
                        sfog = emit_sig(quad)
                        emit_cell_pre(sfog, c_st, sl, nt)
                        emit_cell_post(sfog, c_st, h_st, sl, nt)

            def emit_step(t):
                emit_layer(0)
                if OPTS["proj_delay"] and t > 0:
                    emit_proj(t - 1)
                emit_layer(1)
                if not OPTS["proj_delay"]:
                    emit_proj(t)

            def emit_proj(t):
                # projection: po[3, TN] = W_hp @ h1 tile (stationary whpT)
                out_sb = opool_sb.tile([3, BL], F32, name="out_sb",
                                       tag="out_sb")
                for nt in range(NT):
                    sl = slice(nt * TN, (nt + 1) * TN)
                    po = qpool.tile([3, TN], F32, name="po", tag="quad")
                    nc.tensor.matmul(po[:], whpT[:], h_1[:, sl],
                                     start=True, stop=True)
                    nc.vector.tensor_copy(out_sb[:, sl], po[:])
                nc.sync.dma_start(out=d_out[t], in_=out_sb[:])

            if loop:
                with tc.For_i(0, reps, 1,
                              hint_engines=(mybir.EngineType.PE,)):
                    for t in range(DSTEPS):
                        emit_step(t)
                    if OPTS["proj_delay"]:
                        emit_proj(DSTEPS - 1)
            else:
                for t in range(DSTEPS):
                    emit_step(t)
                if OPTS["proj_delay"]:
                    emit_proj(DSTEPS - 1)

    nc.compile()
    return nc


def _get_compiled(reps=1, loop=False):
    key = (reps, loop)
    if key not in _COMPILED:
        _COMPILED[key] = build_bass(reps=reps, loop=loop)
    return _COMPILED[key]


# ---------------------------------------------------------------------------
# Cached-jit SPMD runner (no donation; inputs stay device-resident)
# ---------------------------------------------------------------------------

class _Runner:
    def __init__(self, nc, n_cores):
        import jax
        from jax.experimental.shard_map import shard_map
        from jax.sharding import Mesh, PartitionSpec
        from concourse import bass2jax

        bass2jax.install_neuronx_cc_hook()
        self.jax = jax
        self.n_cores = n_cores
        partition_name = (
            nc.partition_id_tensor.name if nc.partition_id_tensor else None
        )

        in_names, out_names, out_avals, zero_outs = [], [], [], []
        for alloc in nc.m.functions[0].allocations:
            if not isinstance(alloc, mybir.MemoryLocationSet):
                continue
            name = alloc.memorylocations[0].name
            if alloc.kind == "ExternalInput":
                if name != partition_name:
                    in_names.append(name)
            elif alloc.kind == "ExternalOutput":
                out_names.append(name)
                shape = tuple(alloc.tensor_shape)
                dtype = mybir.dt.np(alloc.dtype)
                out_avals.append(jax.core.ShapedArray(shape, dtype))
                zero_outs.append(np.zeros(shape, dtype))

        n_params = len(in_names)
        all_in_names = list(in_names) + list(out_names)
        if partition_name is not None:
            all_in_names.append(partition_name)

        self.in_names = in_names
        self.out_names = out_names
        self.out_avals = out_avals
        self.zero_outs = zero_outs

        def _body(*args):
            operands = list(args)
            if partition_name is not None:
                operands.append(bass2jax.partition_id_tensor())
            outs = bass2jax._bass_exec_p.bind(
                *operands,
                out_avals=tuple(out_avals),
                in_names=tuple(all_in_names),
                out_names=tuple(out_names),
                lowering_input_output_aliases=(),
                sim_require_finite=True,
                sim_require_nnan=True,
                nc=nc,
            )
            return tuple(outs)

        devices = jax.devices()[:n_cores]
        mesh = Mesh(np.asarray(devices), ("core",))
        n_outs = len(out_avals)
        in_specs = (PartitionSpec("core"),) * (n_params + n_outs)
        out_specs = (PartitionSpec("core"),) * n_outs
        self.fn = jax.jit(
            shard_map(_body, mesh=mesh, in_specs=in_specs,
                      out_specs=out_specs, check_rep=False),
            keep_unused=True,
        )
        self.sharding = jax.sharding.NamedSharding(mesh, PartitionSpec("core"))

    def put_inputs(self, in_maps):
        """Ship per-core inputs (concat on axis 0) + zero outputs to devices."""
        arrs = [
            np.concatenate([np.asarray(m[name]) for m in in_maps], axis=0)
            for name in self.in_names
        ]
        arrs += [
            np.zeros((self.n_cores * z.shape[0], *z.shape[1:]), z.dtype)
            for z in self.zero_outs
        ]
        return [self.jax.device_put(a, self.sharding) for a in arrs]

    def launch(self, dev_args):
        return self.fn(*dev_args)

    def fetch(self, out_arrs):
        res = []
        for c in range(self.n_cores):
            res.append({
                name: np.asarray(out_arrs[i]).reshape(
                    self.n_cores, *self.out_avals[i].shape)[c]
                for i, name in enumerate(self.out_names)
            })
        return res

    def run(self, in_maps, tries=5):
        import time as _time
        dev = self.put_inputs(in_maps)
        last = None
        for a in range(tries):
            try:
                outs = self.launch(dev)
                self.jax.block_until_ready(outs)
                return self.fetch(outs)
            except Exception as e:  # transient tunnel/terminal failures
                last = e
                _time.sleep(2.0)
        raise last


def get_runner(reps=1, loop=False):
    key = (reps, loop)
    if key not in _RUNNERS:
        _RUNNERS[key] = _Runner(_get_compiled(reps=reps, loop=loop), N_CORES)
    return _RUNNERS[key]


# ---------------------------------------------------------------------------
# Host-side prep: permutations, step 0, gather
# ---------------------------------------------------------------------------

def _permg(w):
    """Permute gate-chunk rows of a [512, ...] weight (or [512] bias) to
    kernel chunk order and DOUBLE the g chunk (tanh-via-sigmoid)."""
    if OPTS["split_sig"]:
        perm, gpos = (0, 1, 2, 3), 2      # (i,f,g,o)
    else:
        perm, gpos = (0, 1, 3, 2), 3      # (i,f,o,g)
    wp = w.reshape(4, H, *w.shape[1:])[list(perm)].copy()
    wp[gpos] *= 2.0
    return wp.reshape(w.shape)


def _bias_rows(b):
    """[512] bias -> [128,128] tile with chunk j (kernel order) at row 32j."""
    out = np.zeros((H, H), np.float32)
    bp = _permg(b)
    for j in range(4):
        out[32 * j] = bp[j * H:(j + 1) * H]
    return out


def _sigmoid(x):
    return 1.0 / (1.0 + np.exp(-x))


def _host_step0(obs_traj_rel, h0, c0, W_ih0, W_hh0, b_ih0, b_hh0,
                W_ih1, W_hh1, b_ih1, b_hh1, W_hp, b_hp):
    """Run reference step 0 in numpy fp32; return post-step state + out0."""
    f = np.float32
    x = obs_traj_rel[-1].astype(f)                      # [B, 3]
    hs = [h0[0].astype(f), h0[1].astype(f)]
    cs = [c0[0].astype(f), c0[1].astype(f)]
    params = ((W_ih0, W_hh0, b_ih0, b_hh0), (W_ih1, W_hh1, b_ih1, b_hh1))
    inp = x
    for l, (Wi, Wh, bi, bh) in enumerate(params):
        gates = inp @ Wi.T.astype(f) + hs[l] @ Wh.T.astype(f) \
            + bi.astype(f) + bh.astype(f)
        i, fg, g, o = np.split(gates, 4, axis=-1)
        c_new = _sigmoid(fg) * cs[l] + _sigmoid(i) * np.tanh(g)
        h_new = _sigmoid(o) * np.tanh(c_new)
        hs[l], cs[l] = h_new, c_new
        inp = h_new
    out0 = inp @ W_hp.T.astype(f) + b_hp.astype(f)      # [B, 3]
    return hs, cs, out0


def prep_inputs(obs_traj_rel, h0, c0, W_ih0, W_hh0, b_ih0, b_hh0,
                W_ih1, W_hh1, b_ih1, b_hh1, W_hp, b_hp):
    f = np.float32
    asc = np.ascontiguousarray

    hs, cs, out0 = _host_step0(
        obs_traj_rel, h0, c0, W_ih0, W_hh0, b_ih0, b_hh0,
        W_ih1, W_hh1, b_ih1, b_hh1, W_hp, b_hp)

    W_eff = (W_ih0.astype(f) @ W_hp.astype(f)).astype(f)       # [512, 128]
    b0_eff = (b_ih0 + b_hh0 + W_ih0 @ b_hp).astype(f)
    b1v = (b_ih1 + b_hh1).astype(f)

    shared = {
        "weff": asc(_permg(W_eff).T),             # [128, 512]
        "whh0": asc(_permg(W_hh0.astype(f)).T),
        "wih1": asc(_permg(W_ih1.astype(f)).T),
        "whh1": asc(_permg(W_hh1.astype(f)).T),
        "whpT": asc(W_hp.T.astype(f)),            # [128, 3]
        "b0": _bias_rows(b0_eff),
        "b1": _bias_rows(b1v),
        "ones": np.ones((H, TN), f),
    }

    h0T = hs[0].T       # [128, 8192]
    h1T = hs[1].T
    c0T = cs[0].T
    c1T = cs[1].T

    in_maps = []
    for c in range(N_CORES):
        bsl = slice(c * BL, (c + 1) * BL)
        m = dict(shared)
        m["h0T"] = asc(h0T[:, bsl].astype(f))
        m["h1T"] = asc(h1T[:, bsl].astype(f))
        m["c0T"] = asc(c0T[:, bsl].astype(f))
        m["c1T"] = asc(c1T[:, bsl].astype(f))
        in_maps.append(m)
    return in_maps, out0


def gather(res_results, out0, b_hp):
    # per-core preds [29, 3, 1024]; global step t = device step t-1
    dev = np.concatenate([res_results[c]["preds"] for c in range(N_CORES)],
                         axis=2)                       # [29, 3, 8192]
    preds = np.empty((PRED, B, 3), np.float32)
    preds[0] = out0
    preds[1:] = dev.transpose(0, 2, 1) + b_hp.astype(np.float32)
    return preds


def kernel(**inputs):
    inputs = {k: np.asarray(v) for k, v in inputs.items()}
    in_maps, out0 = prep_inputs(**inputs)
    r = get_runner(reps=1, loop=False)
    res = r.run(in_maps)
    return gather(res, out0, inputs["b_hp"])
